# revision 11
# baseline (speedup 1.0000x reference)
"""Trainium2 Bass kernel for ClipPairWiseLossAll.

loss = sum_{i<j} || relu(r_i - r_j) ||_2   with r = repr[GT], M=512, N=768.

Strategy (8 NeuronCores, SPMD, one shared NEFF):
  * Host: gather r = repr[GT], transpose -> rT [N=768, M=512], cast bf16.
  * Pair space decomposed by DIAGONALS: diagonal o covers pairs (t, t+o),
    t in [0, 512-o). Core c owns o in {16k + (c+1), 16k + (16-c)}, k<32 —
    511 real diagonals + 1 masked dummy, ~16.4k pairs per core.
  * The per-core shift lives in the DATA, not the program: core c receives
    rtab = [rT shifted left by c+1, rT shifted left by 16-c] so the device
    always slices at offset 16k (uniform across cores -> single NEFF).
  * Per k (two diagonals of rounded length L = 512-16k, all 6 n-chunks and
    both slots in single instructions):
      d  = rt2[., t] - rtab[., 16k+t]   one tensor_tensor sub (bf16 2x)
      E  = relu(d)                      one tensor_scalar max-imm (bf16 4x)
      E2 = E^2 -> fp8                   one ACT Square
      psum[row m] += sum_n E2           fp8 DoubleRow one-hot matmuls
  * A per-core mask kills rounded-up columns, ACT computes sqrt with a
    fused row-sum, host adds the 8x64 partials.
"""

import numpy as np

M = 512
N = 768
P = 128
NCH = N // P  # 6
NCORES = 8
NS = 64  # diagonals per core (2 per k)


def _o_list(c):
    out = []
    for k in range(32):
        out.append(16 * k + c + 1)
        out.append(16 * k + 16 - c)
    return out


_PROG = {}

# square-pass engine per k: "act" or "dve" (dve -> bf16 e2, bf16 matmuls)
SQ_DVE_KS = ()


def _build_program():
    if "nc" in _PROG:
        return _PROG["nc"]

    from contextlib import ExitStack

    import concourse.bacc as bacc
    import concourse.tile as tile
    from concourse import mybir

    AOT = mybir.AluOpType
    AFT = mybir.ActivationFunctionType
    bf16 = mybir.dt.bfloat16
    fp8 = mybir.dt.float8e4
    f32 = mybir.dt.float32

    nc = bacc.Bacc(
        "TRN2",
        target_bir_lowering=False,
        debug=False,
        enable_asserts=False,
        num_devices=NCORES,
    )

    rt2_d = nc.dram_tensor("rt2", [P, 2 * NCH * M], bf16, kind="ExternalInput")
    rtab_d = nc.dram_tensor("rtab", [P, 2 * NCH * M], bf16, kind="ExternalInput")
    mk_d = nc.dram_tensor("mk", [NS, M], f32, kind="ExternalInput")
    oh_d = nc.dram_tensor("oh", [P, NS * 2 * NS], fp8, kind="ExternalInput")
    out_d = nc.dram_tensor("out", [NS, 1], f32, kind="ExternalOutput")

    with ExitStack() as ctx:
        tc = ctx.enter_context(tile.TileContext(nc))
        singles = ctx.enter_context(tc.tile_pool(name="singles", bufs=1))
        dpool = ctx.enter_context(tc.tile_pool(name="d", bufs=3))
        epool = ctx.enter_context(tc.tile_pool(name="e", bufs=3))
        e2pool = ctx.enter_context(tc.tile_pool(name="e2", bufs=3))
        pspool = ctx.enter_context(tc.tile_pool(name="ps", bufs=1, space="PSUM"))

        rt2_sb = singles.tile([P, 2, NCH, M], bf16)
        nc.sync.dma_start(out=rt2_sb, in_=rt2_d.ap())
        rtab_sb = singles.tile([P, 2, NCH, M], bf16)
        nc.sync.dma_start(out=rtab_sb, in_=rtab_d.ap())
        mk_sb = singles.tile([NS, M], f32)
        nc.sync.dma_start(out=mk_sb, in_=mk_d.ap())
        oh = singles.tile([P, NS, 2, NS], fp8)
        nc.sync.dma_start(out=oh, in_=oh_d.ap())

        ps = pspool.tile([NS, M], f32)
        nc.vector.memset(ps, 0.0)

        for k in range(32):
            L = M - 16 * k
            d_t = dpool.tile([P, 2, NCH, M], bf16, tag="d")
            nc.vector.tensor_sub(
                d_t[:, :, :, 0:L],
                rt2_sb[:, :, :, 0:L],
                rtab_sb[:, :, :, 16 * k : 16 * k + L],
            )
            e_t = epool.tile([P, 2, NCH, M], bf16, tag="e")
            nc.vector.tensor_scalar(
                out=e_t[:, :, :, 0:L],
                in0=d_t[:, :, :, 0:L],
                scalar1=0.0,
                scalar2=None,
                op0=AOT.max,
            )
            e2_t = e2pool.tile([P, 2, NCH, M], fp8, tag="e2")
            nc.scalar.activation(
                out=e2_t[:, :, :, 0:L],
                in_=e_t[:, :, :, 0:L],
                func=AFT.Square,
            )
            for slot in range(2):
                m = 2 * k + slot
                for c2 in range(NCH // 2):
                    nc.tensor.matmul(
                        ps[:, 0:L],
                        oh[:, m, :, :],
                        e2_t[:, slot, 2 * c2 : 2 * c2 + 2, 0:L],
                        start=False,
                        stop=False,
                        skip_group_check=True,
                        perf_mode=mybir.MatmulPerfMode.DoubleRow,
                    )

        masked = singles.tile([NS, M], f32)
        nc.vector.tensor_mul(masked, ps[:, :], mk_sb)
        sqrt_t = singles.tile([NS, M], bf16)
        res = singles.tile([NS, 1], f32)
        nc.scalar.activation(out=sqrt_t, in_=masked, func=AFT.Sqrt, accum_out=res)
        nc.sync.dma_start(out=out_d.ap(), in_=res)

    nc.compile()
    _PROG["nc"] = nc
    return nc


def _shift_pc(rT_bf, h):
    """rT shifted left by h columns, zero padded, in [p, chunk, t] layout."""
    N_, M_ = rT_bf.shape
    sh = np.zeros_like(rT_bf)
    if h < M_:
        sh[:, : M_ - h] = rT_bf[:, h:]
    return np.transpose(sh.reshape(NCH, P, M_), (1, 0, 2))  # [P, NCH, M]


def _in_maps(repr_np, GT_np):
    import ml_dtypes

    r = np.asarray(repr_np, dtype=np.float32)[np.asarray(GT_np).astype(np.int64)]
    rT = np.ascontiguousarray(r.T)  # [N, M] f32
    rT_bf = rT.astype(ml_dtypes.bfloat16)

    base = _shift_pc(rT_bf, 0)  # [P, NCH, M]
    rt2 = np.ascontiguousarray(
        np.broadcast_to(base[:, None], (P, 2, NCH, M))
    ).reshape(P, -1)

    ohs = np.zeros((P, NS, 2, NS), dtype=ml_dtypes.float8_e4m3)
    for m in range(NS):
        ohs[:, m, :, m] = 1.0
    ohs = ohs.reshape(P, NS * 2 * NS)

    maps = []
    t_idx = np.arange(M)[None, :]
    for c in range(NCORES):
        o = np.array(_o_list(c))
        rtab = np.stack(
            [_shift_pc(rT_bf, c + 1), _shift_pc(rT_bf, 16 - c)], axis=1
        ).reshape(P, -1)
        mk = (t_idx < (M - o)[:, None]).astype(np.float32)  # [64, M]
        maps.append(
            {"rt2": rt2, "rtab": np.ascontiguousarray(rtab), "mk": mk, "oh": ohs}
        )
    return maps


def run_device(repr_np, GT_np, trace=False, trace_cores=None):
    """Run the bass kernel on 8 cores; returns (total, BassKernelResults)."""
    from concourse.bass_utils import run_bass_kernel_spmd

    nc = _build_program()
    maps = _in_maps(repr_np, GT_np)
    res = run_bass_kernel_spmd(
        nc,
        maps,
        core_ids=list(range(NCORES)),
        trace=trace,
        trace_cores=trace_cores,
    )
    total = 0.0
    for core_out in res.results:
        total += float(core_out["out"].astype(np.float64).sum())
    return np.float32(total), res


def kernel(repr, GT):
    total, _ = run_device(repr, GT, trace=False)
    return total


# revision 13
# speedup vs baseline: 1.0217x; 1.0217x over previous
"""Trainium2 Bass kernel for ClipPairWiseLossAll.

loss = sum_{i<j} || relu(r_i - r_j) ||_2   with r = repr[GT], M=512, N=768.

Strategy (8 NeuronCores, SPMD, one shared NEFF):
  * Host: gather r = repr[GT], transpose -> rT [N=768, M=512], cast bf16.
  * Pair space decomposed by DIAGONALS: diagonal o covers pairs (t, t+o),
    t in [0, 512-o). Core c owns o in {16k + (c+1), 16k + (16-c)}, k<32 —
    511 real diagonals + 1 masked dummy, ~16.4k pairs per core.
  * The per-core shift lives in the DATA, not the program: core c receives
    rtab = [rT shifted left by c+1, rT shifted left by 16-c] so the device
    always slices at offset 16k (uniform across cores -> single NEFF).
  * Per k (two diagonals of rounded length L = 512-16k, all 6 n-chunks and
    both slots in single instructions):
      d  = rt2[., t] - rtab[., 16k+t]   one tensor_tensor sub (bf16 2x)
      E  = relu(d)                      one tensor_scalar max-imm (bf16 4x)
      E2 = E^2 -> fp8                   one ACT Square
      psum[row m] += sum_n E2           fp8 DoubleRow one-hot matmuls
  * A per-core mask kills rounded-up columns, ACT computes sqrt with a
    fused row-sum, host adds the 8x64 partials.
"""

import numpy as np

M = 512
N = 768
P = 128
NCH = N // P  # 6
NCORES = 8
NS = 64  # diagonals per core (2 per k)


def _o_list(c):
    out = []
    for k in range(32):
        out.append(16 * k + c + 1)
        out.append(16 * k + 16 - c)
    return out


_PROG = {}

# square-pass engine per k: "act" or "dve" (dve -> bf16 e2, bf16 matmuls)
SQ_DVE_KS = ()


def _build_program():
    if "nc" in _PROG:
        return _PROG["nc"]

    from contextlib import ExitStack

    import concourse.bacc as bacc
    import concourse.tile as tile
    from concourse import mybir

    AOT = mybir.AluOpType
    AFT = mybir.ActivationFunctionType
    bf16 = mybir.dt.bfloat16
    fp8 = mybir.dt.float8e4
    f32 = mybir.dt.float32

    nc = bacc.Bacc(
        "TRN2",
        target_bir_lowering=False,
        debug=False,
        enable_asserts=False,
        num_devices=NCORES,
    )

    rt2_d = nc.dram_tensor("rt2", [P, 2 * NCH * M], bf16, kind="ExternalInput")
    rtab_d = nc.dram_tensor("rtab", [P, 2 * NCH * M], bf16, kind="ExternalInput")
    mk_d = nc.dram_tensor("mk", [NS, M], f32, kind="ExternalInput")
    oh_d = nc.dram_tensor("oh", [P, NS * 2 * NS], fp8, kind="ExternalInput")
    out_d = nc.dram_tensor("out", [NS, 1], f32, kind="ExternalOutput")

    with ExitStack() as ctx:
        tc = ctx.enter_context(tile.TileContext(nc))
        singles = ctx.enter_context(tc.tile_pool(name="singles", bufs=1))
        dpool = ctx.enter_context(tc.tile_pool(name="d", bufs=4))
        epool = ctx.enter_context(tc.tile_pool(name="e", bufs=4))
        e2pool = ctx.enter_context(tc.tile_pool(name="e2", bufs=4))
        pspool = ctx.enter_context(tc.tile_pool(name="ps", bufs=1, space="PSUM"))

        rt2_sb = singles.tile([P, 2, NCH, M], bf16)
        nc.sync.dma_start(out=rt2_sb, in_=rt2_d.ap())
        rtab_sb = singles.tile([P, 2, NCH, M], bf16)
        nc.sync.dma_start(out=rtab_sb, in_=rtab_d.ap())
        mk_sb = singles.tile([NS, M], f32)
        nc.sync.dma_start(out=mk_sb, in_=mk_d.ap())
        oh = singles.tile([P, NS, 2, NS], fp8)
        nc.sync.dma_start(out=oh, in_=oh_d.ap())

        ps = pspool.tile([NS, M], f32)
        nc.vector.memset(ps, 0.0)

        for k in range(32):
            L = M - 16 * k
            d_t = dpool.tile([P, 2, NCH, M], bf16, tag="d")
            nc.vector.tensor_sub(
                d_t[:, :, :, 0:L],
                rt2_sb[:, :, :, 0:L],
                rtab_sb[:, :, :, 16 * k : 16 * k + L],
            )
            e_t = epool.tile([P, 2, NCH, M], bf16, tag="e")
            nc.vector.tensor_scalar(
                out=e_t[:, :, :, 0:L],
                in0=d_t[:, :, :, 0:L],
                scalar1=0.0,
                scalar2=None,
                op0=AOT.max,
            )
            e2_t = e2pool.tile([P, 2, NCH, M], fp8, tag="e2")
            nc.scalar.activation(
                out=e2_t[:, :, :, 0:L],
                in_=e_t[:, :, :, 0:L],
                func=AFT.Square,
            )
            for slot in range(2):
                m = 2 * k + slot
                for c2 in range(NCH // 2):
                    nc.tensor.matmul(
                        ps[:, 0:L],
                        oh[:, m, :, :],
                        e2_t[:, slot, 2 * c2 : 2 * c2 + 2, 0:L],
                        start=False,
                        stop=False,
                        skip_group_check=True,
                        perf_mode=mybir.MatmulPerfMode.DoubleRow,
                    )

        masked = singles.tile([NS, M], f32)
        nc.vector.tensor_mul(masked, ps[:, :], mk_sb)
        sqrt_t = singles.tile([NS, M], bf16)
        res = singles.tile([NS, 1], f32)
        nc.scalar.activation(out=sqrt_t, in_=masked, func=AFT.Sqrt, accum_out=res)
        nc.sync.dma_start(out=out_d.ap(), in_=res)

    nc.compile()
    _PROG["nc"] = nc
    return nc


def _shift_pc(rT_bf, h):
    """rT shifted left by h columns, zero padded, in [p, chunk, t] layout."""
    N_, M_ = rT_bf.shape
    sh = np.zeros_like(rT_bf)
    if h < M_:
        sh[:, : M_ - h] = rT_bf[:, h:]
    return np.transpose(sh.reshape(NCH, P, M_), (1, 0, 2))  # [P, NCH, M]


def _in_maps(repr_np, GT_np):
    import ml_dtypes

    r = np.asarray(repr_np, dtype=np.float32)[np.asarray(GT_np).astype(np.int64)]
    rT = np.ascontiguousarray(r.T)  # [N, M] f32
    rT_bf = rT.astype(ml_dtypes.bfloat16)

    base = _shift_pc(rT_bf, 0)  # [P, NCH, M]
    rt2 = np.ascontiguousarray(
        np.broadcast_to(base[:, None], (P, 2, NCH, M))
    ).reshape(P, -1)

    ohs = np.zeros((P, NS, 2, NS), dtype=ml_dtypes.float8_e4m3)
    for m in range(NS):
        ohs[:, m, :, m] = 1.0
    ohs = ohs.reshape(P, NS * 2 * NS)

    maps = []
    t_idx = np.arange(M)[None, :]
    for c in range(NCORES):
        o = np.array(_o_list(c))
        rtab = np.stack(
            [_shift_pc(rT_bf, c + 1), _shift_pc(rT_bf, 16 - c)], axis=1
        ).reshape(P, -1)
        mk = (t_idx < (M - o)[:, None]).astype(np.float32)  # [64, M]
        maps.append(
            {"rt2": rt2, "rtab": np.ascontiguousarray(rtab), "mk": mk, "oh": ohs}
        )
    return maps


def run_device(repr_np, GT_np, trace=False, trace_cores=None):
    """Run the bass kernel on 8 cores; returns (total, BassKernelResults)."""
    from concourse.bass_utils import run_bass_kernel_spmd

    nc = _build_program()
    maps = _in_maps(repr_np, GT_np)
    res = run_bass_kernel_spmd(
        nc,
        maps,
        core_ids=list(range(NCORES)),
        trace=trace,
        trace_cores=trace_cores,
    )
    total = 0.0
    for core_out in res.results:
        total += float(core_out["out"].astype(np.float64).sum())
    return np.float32(total), res


def kernel(repr, GT):
    total, _ = run_device(repr, GT, trace=False)
    return total
